# revision 1
# baseline (speedup 1.0000x reference)
"""Trainium2 Bass kernel for nn_ComposedCliffordSteerableKernel.

Computation (see reference): for each of 16x16 (m, n) block pairs, a tiny
3D conv (8,8,7^3) x (8,8,7^3) -> (8,8,7^3) with SAME padding, then
elementwise * shell * factor.

Both conv operands depend on the pair, so each pair is an independent
[M=8, K=8, N] matmul per spatial tap -- far too small for the 128x128 PE
array on its own.  Two packings are implemented:

- "f32r"/"f32" (_build_nc): per m-block (8 output rows), one 128x128
  block-diagonal matmul per tap: contraction partitions (n,j) = 16 pairs
  x 8 input blades, output partitions (n,q), free dim = spatial output
  positions of one batch-blade p (N=392, w padded to 8 for FP32R's even
  innermost-run rule).  8 PSUM banks (one per p) accumulate all 343
  taps.  float32r gives single-pass fp32 (1 cycle/row at N>=256) at
  ~tf32 precision (measured 1.4e-4 rel).

- "*t16" (_build_nc_t16): the PE is packed as 16 independent 32x32
  tiles.  Tile (row 32g, col 32c) contracts pair-group g (4 pairs) and
  writes PSUM strip c; pairing c = (g + t) % 4 over tap-classes
  t = lin % 4 uses all 16 tiles and quadruples useful MAC rate vs the
  block-diagonal scheme.  Per output depth od, 4 PSUM banks (one per
  class, od-parity double-buffered) accumulate the taps; output strip s
  is then sum over t of bank_t[strip (s+t)%4] (partition-crossed DVE
  adds).  Zero-contribution (od,kd) pairs are skipped and oh is
  restricted to its valid window (~1.75x fewer MACs).
  Multi-pass modes sweep pass-major so consecutive PE instructions hit
  different tiles (PE matmul starts are pc-monotone; per-tile pass
  chains would collapse the packing to ~1.5x).
  dtypes: "fp16t16" 1-pass fp16 (~3e-4 rel); "bf16t16" 1-pass bf16
  (~2e-3); "bf16x3t16" hi/lo-split 3-pass bf16 (~4e-6, fp32-grade).

k1 is held transposed (columns -> partitions) and zero-padded to
(13,13,14) so every tap is just an AP window offset; weights are
DMA-scattered into block-diagonal SBUF tiles whose off-diagonal zeros
persist from a one-time fill.  Sharding: core c takes output row-blocks
2c and 2c+1; no inter-core communication.
"""

import sys

for _p in ("/opt/trn_rl_repo",):
    if _p not in sys.path:
        sys.path.insert(0, _p)

import numpy as np

NB = 8
KS = 7
S3 = KS * KS * KS          # 343
WPAD = KS + 1              # 8 (even innermost run for fp32r)
SP = KS * KS * WPAD        # 392 psum free size per batch-blade
DPAD, HPAD, WPAD2 = 13, 13, 14
PADVOL = DPAD * HPAD * WPAD2   # 2366 per batch-blade in k1T
N_CORES = 8
M_PER_CORE = 2             # m-blocks per core

# All HW-validated (rel err to reference / notes):
#   "bf16x3t16": 4.3e-6, 16-tile packed PE, 3-pass hi/lo bf16  <- default
#   "fp16t16":   2.9e-4, 16-tile packed PE, fastest
#   "f32r":      1.4e-4, single 128x128 block-diag matmuls
#   "f32":       exact fp32 (4 cycles/row), slowest
MODE = "bf16x3t16"

_CACHE = {}

SPT = KS * WPAD * NB       # 448: T16 psum free per od: (p, oh, ow8)


def _build_nc(mode):
    import concourse.bass as bass
    import concourse.tile as tile
    from concourse import bacc, mybir

    f32 = mybir.dt.float32
    f32r = mybir.dt.float32r
    mult = mybir.AluOpType.mult

    nc = bacc.Bacc("TRN2", target_bir_lowering=False, debug=False)

    # k1 arrives host-padded: [16 rows, 128 cols, 13*13*14] with the 7^3
    # interior at [3:10,3:10,3:10] (f32r tiles cannot be memset, so the
    # zero padding comes in via the cast DMA)
    k1 = nc.dram_tensor(
        "k1pad", [M_PER_CORE * NB, 128, PADVOL], f32, kind="ExternalInput"
    )
    k2 = nc.dram_tensor("k2", [M_PER_CORE * NB, 128, S3], f32, kind="ExternalInput")
    shell = nc.dram_tensor(
        "shell", [M_PER_CORE * NB, 128, SP], f32, kind="ExternalInput"
    )
    factor = nc.dram_tensor("factor", [128, 1], f32, kind="ExternalInput")
    zeros = nc.dram_tensor(
        "zeros", [128, 128 * KS * KS], f32, kind="ExternalInput"
    )
    out = nc.dram_tensor("out", [M_PER_CORE * NB, 128, SP], f32, kind="ExternalOutput")

    mm_dt = f32r if mode == "f32r" else f32

    with tile.TileContext(nc) as tc:
        with (
            tc.tile_pool(name="persist", bufs=1) as persist,
            tc.tile_pool(name="io", bufs=2) as io,
            tc.tile_pool(name="ps", bufs=1, space="PSUM") as pspool,
        ):
            # k1 transposed + zero padded: [(n,j)=128, p=8, 13, 13, 14]
            # stored as float32r so fp32r matmuls accept it (DMA casts)
            k1t = persist.tile([128, NB, DPAD, HPAD, WPAD2], mm_dt, tag="k1t")

            # two weight chunk slots, each one kd-plane of 49 taps:
            # [(n,j)=128, (n,q)=128, tap=49] (taps contiguous so the k2
            # DMA has a stride-1 final dim); zeros off the diagonal persist
            # from a one-time cast-DMA fill from the zeros input
            wslots = []
            for i in range(2):
                w = persist.tile([128, 128, KS * KS], mm_dt, tag=f"w{i}", name=f"w{i}")
                nc.gpsimd.dma_start(
                    out=w.rearrange("c a t -> c (a t)"), in_=zeros[:, :]
                )
                wslots.append(w)

            fac = persist.tile([128, 1], f32, tag="fac")
            nc.sync.dma_start(out=fac[:, :], in_=factor[:, :])

            psum = [
                pspool.tile([128, SP], f32, tag=f"pp{p}", name=f"pp{p}")
                for p in range(NB)
            ]

            for m in range(M_PER_CORE):
                # load k1 block (host-padded, transposed into partitions);
                # one contiguous cast DMA per batch-blade p
                for p in range(NB):
                    nc.gpsimd.dma_start(
                        out=k1t[:, p, :, :, :],
                        in_=k1[m * NB + p, :, :],
                    )

                # shell for this m (host pre-padded w->8, so contiguous),
                # pre-scaled by factor
                sh = io.tile([128, NB, SP], f32, tag="shell")
                nc.sync.dma_start(
                    out=sh[:, :, :],
                    in_=shell[m * NB:(m + 1) * NB, :, :].rearrange("p c s -> c p s"),
                )
                shf = io.tile([128, NB, SP], f32, tag="shellf")
                nc.vector.tensor_scalar_mul(shf[:, :, :], sh[:, :, :], fac[:, 0:1])

                for kd in range(KS):
                    w = wslots[kd % 2]
                    # load this kd-plane's 16 diagonal blocks:
                    # w[n*8+j, n*8+q, t] = k2[m*8+q, n*8+j, kd*49+t]
                    for n in range(16):
                        nc.gpsimd.dma_start(
                            out=w[n * NB:(n + 1) * NB, n * NB:(n + 1) * NB, :],
                            in_=k2[
                                m * NB:(m + 1) * NB,
                                n * NB:(n + 1) * NB,
                                kd * KS * KS:(kd + 1) * KS * KS,
                            ].rearrange("q j t -> j q t"),
                        )
                    for kh in range(KS):
                        for kw in range(KS):
                            t = kh * KS + kw
                            lhsT = w[:, :, t]
                            first = kd == 0 and t == 0
                            last = kd == KS - 1 and t == KS * KS - 1
                            for p in range(NB):
                                rhs = k1t[
                                    :, p, kd:kd + KS, kh:kh + KS, kw:kw + WPAD
                                ]
                                nc.tensor.matmul(
                                    psum[p][:, :],
                                    lhsT,
                                    rhs,
                                    start=first,
                                    stop=last,
                                )

                # evacuate: out = psum * factor * shell  (shell already
                # carries factor), then store
                ost = io.tile([128, NB, SP], f32, tag="ost")
                for p in range(NB):
                    nc.vector.tensor_mul(
                        ost[:, p, :], psum[p][:, :], shf[:, p, :]
                    )
                nc.sync.dma_start(
                    out=out[m * NB:(m + 1) * NB, :, :].rearrange("p c s -> c p s"),
                    in_=ost[:, :, :],
                )
    nc.compile()
    return nc


def _build_nc_t16(mode):
    """16x 32x32 PE-tile variant (bf16/fp16).

    Per m-block, per output depth od (7), accumulate the valid taps into
    4 PSUM banks (one per tap-class t = lin%4), double-buffered by od
    parity.  Tile (row 32g, col 32c) contracts pair-group g (SBUF
    partitions 32g..32g+31 of k1t) and writes PSUM partitions 32c;
    pairing c = (g+t)%4 uses all 16 tiles.  Output strip s is then
    sum over t of bank_t[strip (s+t)%4]; partition rotation goes through
    SBUF->SBUF DMA (engines cannot cross partitions).

    Multi-pass modes emit pass-major sweeps: PE matmuls start in program
    order, so back-to-back passes on the SAME tile would serialize and
    collapse the 16-tile concurrency; sweeping all (tap, g) per pass
    keeps consecutive instructions on different tiles.

    psum bank free layout is (oh, p, ow) with ow=7 (no fp32r evenness
    rule here), so an oh-window slice stays a contiguous slab (the sim's
    matmul needs 2D-flattenable psum dst APs).
    """
    import concourse.tile as tile
    from concourse import bacc, mybir

    f32 = mybir.dt.float32
    bf16 = (mybir.dt.float16 if mode == "fp16t16" else mybir.dt.bfloat16)
    npass = 3 if mode == "bf16x3t16" else 1
    SPT7 = KS * KS * NB            # 392: (oh, p, ow7)
    S2 = KS * KS

    nc = bacc.Bacc("TRN2", target_bir_lowering=False, debug=False)

    names = ["h"] if npass == 1 else ["h", "l"]
    k1d = {
        s: nc.dram_tensor(f"k1{s}", [M_PER_CORE * NB, 128, S3], bf16,
                          kind="ExternalInput")
        for s in names
    }
    k2d = {
        s: nc.dram_tensor(f"k2{s}", [M_PER_CORE * NB, 128, S3], bf16,
                          kind="ExternalInput")
        for s in names
    }
    shell = nc.dram_tensor(
        "shell", [M_PER_CORE * NB, 128, S3], f32, kind="ExternalInput"
    )
    factor = nc.dram_tensor("factor", [128, 1], f32, kind="ExternalInput")
    out = nc.dram_tensor("out", [M_PER_CORE * NB, 128, S3], f32,
                         kind="ExternalOutput")

    # (weight-piece, k1-piece) per pass: h*h + h*l + l*h
    passes = [("h", "h")] if npass == 1 else [("h", "h"), ("h", "l"), ("l", "h")]

    with tile.TileContext(nc) as tc:
        with (
            tc.tile_pool(name="persist", bufs=1) as persist,
            tc.tile_pool(name="io", bufs=2) as io,
            tc.tile_pool(name="ps", bufs=1, space="PSUM") as pspool,
        ):
            # k1t: (d, h) padding is never read (the kd-skip keeps
            # od+kd in the interior and the oh-window keeps oh+kh in the
            # interior), so only w carries the zero halo: 9KB/partition
            # per piece instead of 35KB -- leaves room to double-buffer
            # k1t AND weights across m-blocks (no m-boundary PE stall)
            k1t = {
                (s, i): persist.tile([128, NB, KS, KS, DPAD], bf16,
                                     tag=f"k1t{s}{i}", name=f"k1t{s}{i}")
                for s in names for i in range(2)
            }
            for tile_ in k1t.values():
                nc.vector.memset(tile_[:, :, :, :, :], 0.0)

            # weights: [128=(g,nsub,j), 32=(nsub,q), 343 taps] per piece
            nwslot = 2
            wt = {}
            for s in names:
                for i in range(nwslot):
                    w = persist.tile([128, 32, S3], bf16,
                                     tag=f"wt{s}{i}", name=f"wt{s}{i}")
                    nc.vector.memset(w[:, :, :], 0.0)
                    wt[(s, i)] = w

            fac = persist.tile([128, 1], f32, tag="fac")
            nc.sync.dma_start(out=fac[:, :], in_=factor[:, :])

            # psum: [od-parity][class] -> [128, 392] (allocated 400 wide
            # so 32-partition strip offsets stay 2KB-bank aligned:
            # 32*400*4 % 2048 == 0)
            psumb = [
                [
                    pspool.tile([128, 400], f32, tag=f"pb{par}{t}",
                                name=f"pb{par}{t}")[:, 0:SPT7]
                    for t in range(4)
                ]
                for par in range(2)
            ]
            # valid-window skipping leaves some psum elements unwritten
            # in a round (their true partial is 0); a one-time zero fill
            # keeps those reads defined
            for par in range(2):
                for t in range(4):
                    nc.vector.memset(psumb[par][t][:, :], 0.0)

            for m in range(M_PER_CORE):
                k1m = {s: k1t[(s, m % 2)] for s in names}
                for s in names:
                    for p in range(NB):
                        src_p = k1d[s][m * NB + p, :, :].rearrange(
                            "c (d h w) -> c d h w", d=KS, h=KS, w=KS
                        )
                        for d in range(KS):
                            nc.sync.dma_start(
                                out=k1m[s][:, p, d, :, 3:3 + KS],
                                in_=src_p[:, d, :, :],
                            )
                wm = {s: wt[(s, m % nwslot)] for s in names}
                for s in names:
                    for n in range(16):
                        nc.sync.dma_start(
                            out=wm[s][n * NB:(n + 1) * NB,
                                      (n % 4) * NB:(n % 4 + 1) * NB, :],
                            in_=k2d[s][
                                m * NB:(m + 1) * NB, n * NB:(n + 1) * NB, :
                            ].rearrange("q j t -> j q t"),
                        )

                shf = io.tile([128, NB, S3], f32, tag="shell")
                nc.sync.dma_start(
                    out=shf[:, :, :],
                    in_=shell[m * NB:(m + 1) * NB, :, :].rearrange("p c s -> c p s"),
                )
                nc.vector.tensor_scalar_mul(shf[:, :, :], shf[:, :, :], fac[:, 0:1])

                ost = io.tile([128, NB, KS, KS, KS], f32, tag="ost")

                for od in range(KS):
                    par = od % 2
                    # valid windows: contributions are zero unless the
                    # padded read index lands in the 7^3 interior [3,10)
                    kds = [kd for kd in range(KS) if 3 <= od + kd <= 9]
                    # each class t starts with a full-oh tap (kh=3; class
                    # of (kd,3,kw) is (kd+1+kw)%4) so the accumulation
                    # group's first matmul covers the whole bank
                    firsts = []
                    for t in range(4):
                        kd0 = kds[0]
                        kw0 = (t - kd0 - 1) % 4
                        firsts.append(kd0 * S2 + 3 * KS + kw0)
                    assert sorted(l % 4 for l in firsts) == [0, 1, 2, 3]
                    ordered = firsts + [
                        lin
                        for kd in kds
                        for lin in range(kd * S2, (kd + 1) * S2)
                        if lin not in set(firsts)
                    ]
                    last_lin_od = {t: max(l for l in ordered if l % 4 == t)
                                   for t in range(4)}
                    for ip, (ws, ks) in enumerate(passes):
                        for i, lin in enumerate(ordered):
                            kd, r = divmod(lin, S2)
                            kh, kw = divmod(r, KS)
                            oh0, oh1 = max(0, 3 - kh), min(KS, 10 - kh)
                            t = lin % 4
                            first = ip == 0 and i < 4
                            last = ip == npass - 1 and lin == last_lin_od[t]
                            for g in range(4):
                                c = (g + t) % 4
                                dst = psumb[par][t][
                                    32 * c:32 * c + 32, :
                                ].rearrange(
                                    "c (oh p ow) -> c oh p ow", oh=KS, p=NB,
                                )[:, oh0:oh1, :, :]
                                rhs = k1m[ks][
                                    32 * g:32 * g + 32, :,
                                    od + kd - 3,
                                    kh + oh0 - 3:kh + oh1 - 3,
                                    kw:kw + KS,
                                ].transpose([0, 2, 1, 3])  # (oh, p, ow)
                                nc.tensor.matmul(
                                    dst,
                                    wm[ws][32 * g:32 * g + 32, :, lin],
                                    rhs,
                                    start=first,
                                    stop=last,
                                    tile_position=(32 * g, 32 * c),
                                    # sim group-check is per 2KB
                                    # zero-region; per-strip groups are
                                    # safe on HW (num_active_cols=32)
                                    skip_group_check=True,
                                )
                    # combine rotated partials into ost[:, :, od, :, :].
                    # bank 0 is strip-aligned (c = g for t = 0) and is
                    # read from PSUM directly; banks 1-3 go through an
                    # aligned DVE evacuation then a partition-rotating
                    # SBUF->SBUF DMA.
                    ev = {
                        t: io.tile([128, SPT7], f32, tag=f"ev{t}",
                                   name=f"ev{t}")
                        for t in range(1, 4)
                    }
                    for t in range(1, 4):
                        nc.vector.tensor_copy(ev[t][:, :], psumb[par][t][:, :])
                    rt = {}
                    for t in range(1, 4):
                        r = io.tile([128, SPT7], f32, tag=f"rt{t}",
                                    name=f"rt{t}")
                        sh4 = 32 * t
                        nc.sync.dma_start(
                            out=r[0:128 - sh4, :], in_=ev[t][sh4:128, :]
                        )
                        nc.sync.dma_start(
                            out=r[128 - sh4:128, :], in_=ev[t][0:sh4, :]
                        )
                        rt[t] = r
                    o_sl = ost[:, :, od, :, :]
                    fix = lambda ap: ap.rearrange(
                        "c (oh p ow) -> c p oh ow", oh=KS, p=NB
                    )
                    nc.vector.tensor_add(
                        o_sl, fix(psumb[par][0][:, :]), fix(rt[1][:, :])
                    )
                    nc.vector.tensor_add(o_sl, o_sl, fix(rt[2][:, :]))
                    nc.vector.tensor_add(o_sl, o_sl, fix(rt[3][:, :]))

                ostf = ost.rearrange("c p a b w -> c p (a b w)")
                nc.vector.tensor_mul(ostf[:, :, :], ostf[:, :, :], shf[:, :, :])
                nc.sync.dma_start(
                    out=out[m * NB:(m + 1) * NB, :, :].rearrange("p c s -> c p s"),
                    in_=ostf[:, :, :],
                )
    nc.compile()
    return nc


def _get_nc(mode=None):
    if mode is None:
        mode = MODE
    if mode not in _CACHE:
        if mode in ("bf16t16", "bf16x3t16", "fp16t16"):
            _CACHE[mode] = _build_nc_t16(mode)
        else:
            _CACHE[mode] = _build_nc(mode)
    return _CACHE[mode]


def _make_in_maps(k1, k2, shell, factor, mode=None):
    import ml_dtypes

    if mode is None:
        mode = MODE

    k1 = np.ascontiguousarray(k1.reshape(128, 128, S3), np.float32)
    k2 = np.ascontiguousarray(k2.reshape(128, 128, S3), np.float32)
    if mode in ("f32r", "f32"):
        shell_p = np.zeros((128, 128, KS, KS, WPAD), np.float32)
        shell_p[..., :KS] = shell.reshape(128, 128, KS, KS, KS)
        shell_p = shell_p.reshape(128, 128, SP)
    else:
        shell_p = np.ascontiguousarray(shell.reshape(128, 128, S3), np.float32)
    fac = np.full((128, 1), np.float32(factor.reshape(-1)[0]), np.float32)
    rows = M_PER_CORE * NB

    common = {"shell": shell_p, "factor": fac}
    if mode in ("f32r", "f32"):
        k1_pad = np.zeros((128, 128, DPAD, HPAD, WPAD2), np.float32)
        k1_pad[:, :, 3:3 + KS, 3:3 + KS, 3:3 + KS] = k1.reshape(
            128, 128, KS, KS, KS
        )
        k1_pad = k1_pad.reshape(128, 128, PADVOL)
        zeros = np.zeros((128, 128 * KS * KS), np.float32)
        per_full = {"k1pad": k1_pad, "k2": k2, **common}
        shared = {"zeros": zeros}
    else:
        bf = np.float16 if mode == "fp16t16" else ml_dtypes.bfloat16
        k1h = k1.astype(bf)
        k2h = k2.astype(bf)
        per_full = {"k1h": k1h, "k2h": k2h, **common}
        if mode == "bf16x3t16":
            per_full["k1l"] = (k1 - k1h.astype(np.float32)).astype(bf)
            per_full["k2l"] = (k2 - k2h.astype(np.float32)).astype(bf)
        shared = {}

    maps = []
    for c in range(N_CORES):
        m = {k: v[c * rows:(c + 1) * rows] for k, v in per_full.items()
             if k != "factor"}
        m["factor"] = fac
        m.update(shared)
        maps.append(m)
    return maps


def _gather(results):
    outs = [np.asarray(r["out"]) for r in results]
    full = np.concatenate(outs, axis=0)          # (128, 128, 392|343)
    if full.shape[-1] == SP:  # f32r/f32 path: strip the ow pad
        full = full.reshape(128, 128, KS, KS, WPAD)[..., :KS]
        return np.ascontiguousarray(full)
    return full.reshape(128, 128, KS, KS, KS)


def kernel(k1, k2, shell, factor, _trace=False):
    from concourse.bass_utils import run_bass_kernel_spmd

    nc = _get_nc(MODE)
    in_maps = _make_in_maps(
        np.asarray(k1), np.asarray(k2), np.asarray(shell), np.asarray(factor),
        mode=MODE,
    )
    try:
        res = run_bass_kernel_spmd(
            nc, in_maps, core_ids=list(range(N_CORES)), trace=_trace
        )
    except ModuleNotFoundError:
        # no NTFF profiling hook in this container; run without trace
        res = run_bass_kernel_spmd(
            nc, in_maps, core_ids=list(range(N_CORES)), trace=False
        )
    out = _gather(res.results)
    if _trace:
        return out, res
    return out



# revision 3
# speedup vs baseline: 14.8207x; 14.8207x over previous
"""Trainium2 Bass kernel for nn_ComposedCliffordSteerableKernel.

Computation (see reference): for each of 16x16 (m, n) block pairs, a tiny
3D conv (8,8,7^3) x (8,8,7^3) -> (8,8,7^3) with SAME padding, then
elementwise * shell * factor:

  out[(m,p),(n,q),x] = sum_{j,tap} k1[(m,p),(n,j),x+tap-3] k2[(m,q),(n,j),tap]

Sharding: core c takes output row-blocks m = 2c, 2c+1; no inter-core
communication (gather on host).

Packing ("fp16win", default): per m-block, 128x128 block-diagonal
matmuls -- contraction partitions (n,j) = 16 pairs x 8 input blades,
output partitions (n,q).  The pair index n must live in the contraction
partitions (the rhs is shared by all output columns), which caps useful
MACs at 16*8*8 = 1024 per streamed PSUM row; the optimum is therefore to
stream ONLY valid rows.  All three window dims of the conv are clipped:

  - od: PSUM is split per output depth -- bank (od+m)%8 holds the
    (oh, ow, p) = 7*7*8 = 392-float slab for that od; a tap (kd,*,*)
    only touches banks whose od has od+kd-3 in [0,7).
  - oh: one matmul per (tap, od, oh) with oh restricted to its valid
    window; dst = bank[:, oh, ow0:ow1, :] stays a contiguous run.
  - ow: the innermost (ow, p) run is clipped to the valid ow window of
    kw; rhs = k1t[:, d, h, wlo:whi, :] is the matching contiguous run
    (k1t is held un-padded -- every read lands in the 7^3 interior).

Streamed rows/core = 2m * 8p * 37^3 = 810,448 -- every row is a fully
valid output contribution (37 = sum_k (7-|k-3|) per dim).  fp16 keeps
1 cycle/row on the PE and ~3e-4 rel err.  Weights are DMA-scattered
into two kd-plane block-diagonal tiles ([128, 128, 49], off-diagonal
zeros persist from a one-time memset); psum accumulation uses
start=False onto DVE-zeroed banks (skip_group_check).  Bank map
(od + m) keeps the m1 plane-0 banks ones that m0 finished early, so
the PE never stalls at the m boundary.  shell*factor is folded on the
host; DVE fuses psum * shellf during evacuation.
"""

import sys

for _p in ("/opt/trn_rl_repo",):
    if _p not in sys.path:
        sys.path.insert(0, _p)

import numpy as np

NB = 8
KS = 7
S2 = KS * KS               # 49
S3 = KS * KS * KS          # 343
N_CORES = 8
M_PER_CORE = 2             # m-blocks per core
PXS = NB * S3              # 2744: (p, x) free block per m

# Modes (HW-validated rel err):
#   "fp16win": windowed block-diag fp16 (~3e-4)  <- default
#   "bf16win": same scheme in bf16 (~2e-3)
#   "f32r":    full-window block-diag float32r (1.4e-4), prior fallback
MODE = "fp16win"

_CACHE = {}


def _win(k):
    """Valid output range [o0, o1) for kernel offset k: o+k-3 in [0, 7)."""
    return max(0, 3 - k), min(KS, 10 - k)


def _build_nc_win(mode):
    import concourse.tile as tile
    from concourse import bacc, mybir

    f32 = mybir.dt.float32
    f16 = mybir.dt.float16 if mode == "fp16win" else mybir.dt.bfloat16

    nc = bacc.Bacc("TRN2", target_bir_lowering=False, debug=False)

    # host-prearranged inputs (per core):
    #   k1t_d[m, c=(n,j), ((d,h,w), p)]  fp16, transposed + p-innermost
    #   k2w_d[m, kd, n, j, q, t]         fp16, diagonal blocks by kd-plane
    #   shf_d[m, c, p, x]                f32, shell * factor
    k1t_d = nc.dram_tensor("k1t", [M_PER_CORE, 128, S3 * NB], f16,
                           kind="ExternalInput")
    k2w_d = nc.dram_tensor("k2w", [M_PER_CORE, KS, 16, NB, NB, S2], f16,
                           kind="ExternalInput")
    shf_d = nc.dram_tensor("shf", [M_PER_CORE, 128, NB, S3], f32,
                           kind="ExternalInput")
    out_d = nc.dram_tensor("out", [M_PER_CORE, 128, NB, S3], f32,
                           kind="ExternalOutput")

    with tile.TileContext(nc) as tc:
        with (
            tc.tile_pool(name="persist", bufs=1) as persist,
            tc.tile_pool(name="io", bufs=2) as io,
            tc.tile_pool(name="ps", bufs=1, space="PSUM") as pspool,
        ):
            # two kd-plane weight slots; block-diagonal, zeros persist
            wslots = [
                persist.tile([128, 128, S2], f16, tag=f"w{i}", name=f"w{i}")
                for i in range(2)
            ]
            # 8 psum banks, bank-aligned via full-bank tiles
            banks = [
                pspool.tile([128, 512], f32, tag=f"pb{b}", name=f"pb{b}")
                for b in range(8)
            ]
            bank_mm = [
                b[:, 0:392].rearrange("c (oh ow p) -> c oh ow p", oh=KS, ow=KS)
                for b in banks
            ]
            bank_ev = [
                b[:, 0:392].rearrange("c (oh ow p) -> c p oh ow", oh=KS, ow=KS)
                for b in banks
            ]

            # one-time zeroing; w0 + m0's first banks (3..6) first so the
            # kd=0 weight DMA and first matmuls are not held up
            nc.vector.memset(wslots[0][:, :, :], 0.0)
            for b in (3, 4, 5, 6):
                nc.vector.memset(banks[b][:, 0:392], 0.0)
            nc.vector.memset(wslots[1][:, :, :], 0.0)
            for b in (0, 1, 2, 7):
                nc.vector.memset(banks[b][:, 0:392], 0.0)

            for m in range(M_PER_CORE):
                k1t = io.tile([128, KS, KS, KS, NB], f16, tag="k1t")
                nc.sync.dma_start(
                    out=k1t.rearrange("c a b w p -> c (a b w p)"),
                    in_=k1t_d[m, :, :],
                )
                shf = io.tile([128, NB, S3], f32, tag="shf")
                nc.sync.dma_start(out=shf[:, :, :], in_=shf_d[m, :, :, :])
                ost = io.tile([128, NB, S3], f32, tag="ost")
                ost_v = ost.rearrange("c p (od a b) -> c p od a b", od=KS, a=KS)
                shf_v = shf.rearrange("c p (od a b) -> c p od a b", od=KS, a=KS)

                # slot parity follows the global plane counter so the
                # m1 kd=0 load lands in the slot m0's kd=6 is NOT using
                def load_w(kd, m=m):
                    w = wslots[(m * KS + kd) % 2]
                    for n in range(16):
                        nc.sync.dma_start(
                            out=w[n * NB:(n + 1) * NB,
                                  n * NB:(n + 1) * NB, :],
                            in_=k2w_d[m, kd, n, :, :, :],
                        )

                load_w(0)
                for kd in range(KS):
                    if kd + 1 < KS:
                        load_w(kd + 1)
                    w = wslots[(m * KS + kd) % 2]
                    od0, od1 = _win(kd)
                    for kh in range(KS):
                        oh0, oh1 = _win(kh)
                        for kw in range(KS):
                            ow0, ow1 = _win(kw)
                            lhsT = w[:, :, kh * KS + kw]
                            wlo = ow0 + kw - 3
                            whi = ow1 + kw - 3
                            for od in range(od0, od1):
                                dstb = bank_mm[od + m]
                                d = od + kd - 3
                                for oh in range(oh0, oh1):
                                    nc.tensor.matmul(
                                        dstb[:, oh, ow0:ow1, :],
                                        lhsT,
                                        k1t[:, d, oh + kh - 3, wlo:whi, :],
                                        start=False,
                                        stop=False,
                                        skip_group_check=True,
                                    )

                # evacuate in completion order (od 6,5,4 finish at kd
                # 3,4,5; the rest at kd 6); fuse * shellf.  After each
                # evac, re-zero the bank if the next m needs it (m1 uses
                # banks 1..7; bank 7 is still zero from the initial
                # memset).
                for od in (6, 5, 4, 3, 2, 1, 0):
                    b = od + m
                    nc.vector.tensor_mul(
                        ost_v[:, :, od, :, :],
                        bank_ev[b],
                        shf_v[:, :, od, :, :],
                    )
                    if m + 1 < M_PER_CORE and 1 <= b <= 6:
                        nc.vector.memset(banks[b][:, 0:392], 0.0)

                nc.sync.dma_start(out=out_d[m, :, :, :], in_=ost[:, :, :])
    nc.compile()
    return nc


def _build_nc_f32r():
    """Prior fallback: full-window block-diagonal float32r (see git
    history for the original docstring)."""
    import concourse.tile as tile
    from concourse import bacc, mybir

    f32 = mybir.dt.float32
    f32r = mybir.dt.float32r
    WPAD = KS + 1
    SP = KS * KS * WPAD
    DPAD, HPAD, WPAD2 = 13, 13, 14
    PADVOL = DPAD * HPAD * WPAD2

    nc = bacc.Bacc("TRN2", target_bir_lowering=False, debug=False)
    k1 = nc.dram_tensor("k1pad", [M_PER_CORE * NB, 128, PADVOL], f32,
                        kind="ExternalInput")
    k2 = nc.dram_tensor("k2", [M_PER_CORE * NB, 128, S3], f32,
                        kind="ExternalInput")
    shell = nc.dram_tensor("shell", [M_PER_CORE * NB, 128, SP], f32,
                           kind="ExternalInput")
    factor = nc.dram_tensor("factor", [128, 1], f32, kind="ExternalInput")
    zeros = nc.dram_tensor("zeros", [128, 128 * S2], f32, kind="ExternalInput")
    out = nc.dram_tensor("out", [M_PER_CORE * NB, 128, SP], f32,
                         kind="ExternalOutput")

    with tile.TileContext(nc) as tc:
        with (
            tc.tile_pool(name="persist", bufs=1) as persist,
            tc.tile_pool(name="io", bufs=2) as io,
            tc.tile_pool(name="ps", bufs=1, space="PSUM") as pspool,
        ):
            k1t = persist.tile([128, NB, DPAD, HPAD, WPAD2], f32r, tag="k1t")
            wslots = []
            for i in range(2):
                w = persist.tile([128, 128, S2], f32r, tag=f"w{i}",
                                 name=f"w{i}")
                nc.gpsimd.dma_start(out=w.rearrange("c a t -> c (a t)"),
                                    in_=zeros[:, :])
                wslots.append(w)
            fac = persist.tile([128, 1], f32, tag="fac")
            nc.sync.dma_start(out=fac[:, :], in_=factor[:, :])
            psum = [pspool.tile([128, SP], f32, tag=f"pp{p}", name=f"pp{p}")
                    for p in range(NB)]

            for m in range(M_PER_CORE):
                for p in range(NB):
                    nc.gpsimd.dma_start(out=k1t[:, p, :, :, :],
                                        in_=k1[m * NB + p, :, :])
                sh = io.tile([128, NB, SP], f32, tag="shell")
                nc.sync.dma_start(
                    out=sh[:, :, :],
                    in_=shell[m * NB:(m + 1) * NB, :, :].rearrange(
                        "p c s -> c p s"),
                )
                shf = io.tile([128, NB, SP], f32, tag="shellf")
                nc.vector.tensor_scalar_mul(shf[:, :, :], sh[:, :, :],
                                            fac[:, 0:1])
                for kd in range(KS):
                    w = wslots[kd % 2]
                    for n in range(16):
                        nc.gpsimd.dma_start(
                            out=w[n * NB:(n + 1) * NB, n * NB:(n + 1) * NB, :],
                            in_=k2[m * NB:(m + 1) * NB,
                                   n * NB:(n + 1) * NB,
                                   kd * S2:(kd + 1) * S2].rearrange(
                                       "q j t -> j q t"),
                        )
                    for kh in range(KS):
                        for kw in range(KS):
                            t = kh * KS + kw
                            lhsT = w[:, :, t]
                            first = kd == 0 and t == 0
                            last = kd == KS - 1 and t == S2 - 1
                            for p in range(NB):
                                rhs = k1t[:, p, kd:kd + KS, kh:kh + KS,
                                          kw:kw + WPAD]
                                nc.tensor.matmul(psum[p][:, :], lhsT, rhs,
                                                 start=first, stop=last)
                ost = io.tile([128, NB, SP], f32, tag="ost")
                for p in range(NB):
                    nc.vector.tensor_mul(ost[:, p, :], psum[p][:, :],
                                         shf[:, p, :])
                nc.sync.dma_start(
                    out=out[m * NB:(m + 1) * NB, :, :].rearrange(
                        "p c s -> c p s"),
                    in_=ost[:, :, :],
                )
    nc.compile()
    return nc


def _get_nc(mode=None):
    if mode is None:
        mode = MODE
    if mode not in _CACHE:
        if mode in ("fp16win", "bf16win"):
            _CACHE[mode] = _build_nc_win(mode)
        else:
            _CACHE[mode] = _build_nc_f32r()
    return _CACHE[mode]


def _make_in_maps(k1, k2, shell, factor, mode=None):
    if mode is None:
        mode = MODE

    k1 = np.ascontiguousarray(k1.reshape(128, 128, S3), np.float32)
    k2 = np.ascontiguousarray(k2.reshape(128, 128, S3), np.float32)
    shell = shell.reshape(128, 128, S3)
    fval = np.float32(factor.reshape(-1)[0])
    rows = M_PER_CORE * NB

    maps = []
    if mode in ("fp16win", "bf16win"):
        if mode == "bf16win":
            import ml_dtypes
            hdt = ml_dtypes.bfloat16
        else:
            hdt = np.float16
        shellf = (shell * fval).astype(np.float32)
        for c in range(N_CORES):
            sl = slice(c * rows, (c + 1) * rows)
            # k1t[m, c, (x, p)]: rows (m,p) -> free, transposed
            k1c = k1[sl].reshape(M_PER_CORE, NB, 128, S3)
            k1t = np.ascontiguousarray(
                k1c.transpose(0, 2, 3, 1)).astype(hdt).reshape(
                    M_PER_CORE, 128, S3 * NB)
            # k2w[m, kd, n, j, q, t]
            k2c = k2[sl].reshape(M_PER_CORE, NB, 16, NB, KS, S2)
            k2w = np.ascontiguousarray(
                k2c.transpose(0, 4, 2, 3, 1, 5)).astype(hdt)
            # shf[m, c, p, x]
            shc = shellf[sl].reshape(M_PER_CORE, NB, 128, S3)
            shf = np.ascontiguousarray(shc.transpose(0, 2, 1, 3))
            maps.append({"k1t": k1t, "k2w": k2w, "shf": shf})
        return maps

    # f32r fallback path
    WPAD = KS + 1
    SP = KS * KS * WPAD
    DPAD, HPAD, WPAD2 = 13, 13, 14
    shell_p = np.zeros((128, 128, KS, KS, WPAD), np.float32)
    shell_p[..., :KS] = shell.reshape(128, 128, KS, KS, KS)
    shell_p = shell_p.reshape(128, 128, SP)
    fac = np.full((128, 1), fval, np.float32)
    k1_pad = np.zeros((128, 128, DPAD, HPAD, WPAD2), np.float32)
    k1_pad[:, :, 3:3 + KS, 3:3 + KS, 3:3 + KS] = k1.reshape(
        128, 128, KS, KS, KS)
    k1_pad = k1_pad.reshape(128, 128, DPAD * HPAD * WPAD2)
    zeros = np.zeros((128, 128 * S2), np.float32)
    for c in range(N_CORES):
        sl = slice(c * rows, (c + 1) * rows)
        maps.append({
            "k1pad": k1_pad[sl], "k2": k2[sl], "shell": shell_p[sl],
            "factor": fac, "zeros": zeros,
        })
    return maps


def _gather(results, mode):
    outs = [np.asarray(r["out"]) for r in results]
    if mode in ("fp16win", "bf16win"):
        # per core: [m, c, p, x] -> rows (2c+m)*8+p
        full = np.empty((128, 128, S3), np.float32)
        for c, o in enumerate(outs):
            full[c * 16:(c + 1) * 16] = o.transpose(0, 2, 1, 3).reshape(
                16, 128, S3)
        return full.reshape(128, 128, KS, KS, KS)
    full = np.concatenate(outs, axis=0)
    WPAD = KS + 1
    full = full.reshape(128, 128, KS, KS, WPAD)[..., :KS]
    return np.ascontiguousarray(full)


def kernel(k1, k2, shell, factor, _trace=False):
    from concourse.bass_utils import run_bass_kernel_spmd

    nc = _get_nc(MODE)
    in_maps = _make_in_maps(
        np.asarray(k1), np.asarray(k2), np.asarray(shell), np.asarray(factor),
        mode=MODE,
    )
    try:
        res = run_bass_kernel_spmd(
            nc, in_maps, core_ids=list(range(N_CORES)), trace=_trace
        )
    except ModuleNotFoundError:
        res = run_bass_kernel_spmd(
            nc, in_maps, core_ids=list(range(N_CORES)), trace=False
        )
    out = _gather(res.results, MODE)
    if _trace:
        return out, res
    return out


# revision 7
# speedup vs baseline: 15.2900x; 1.0317x over previous
"""Trainium2 Bass kernel for nn_ComposedCliffordSteerableKernel.

Computation (see reference): for each of 16x16 (m, n) block pairs, a tiny
3D conv (8,8,7^3) x (8,8,7^3) -> (8,8,7^3) with SAME padding, then
elementwise * shell * factor:

  out[(m,p),(n,q),x] = sum_{j,tap} k1[(m,p),(n,j),x+tap-3] k2[(m,q),(n,j),tap]

Sharding: core c takes output row-blocks m = 2c, 2c+1; no inter-core
communication (gather on host).

Packing ("fp16win", default): per m-block, 128x128 block-diagonal
matmuls -- contraction partitions (n,j) = 16 pairs x 8 input blades,
output partitions (n,q).  The pair index n must live in the contraction
partitions (the rhs is shared by all output columns), which caps useful
MACs at 16*8*8 = 1024 per streamed PSUM row; the optimum is therefore to
stream ONLY valid rows.  All three window dims of the conv are clipped:

  - od: PSUM is split per output depth -- bank (od+m)%8 holds the
    (oh, ow, p) = 7*7*8 = 392-float slab for that od; a tap (kd,*,*)
    only touches banks whose od has od+kd-3 in [0,7).
  - oh: one matmul per (tap, od, oh) with oh restricted to its valid
    window; dst = bank[:, oh, ow0:ow1, :] stays a contiguous run.
  - ow: the innermost (ow, p) run is clipped to the valid ow window of
    kw; rhs = k1t[:, d, h, wlo:whi, :] is the matching contiguous run
    (k1t is held un-padded -- every read lands in the 7^3 interior).

Streamed rows/core = 2m * 8p * 37^3 = 810,448 -- every row is a fully
valid output contribution (37 = sum_k (7-|k-3|) per dim).  fp16 keeps
1 cycle/row on the PE and ~3e-4 rel err.  Weights are DMA-scattered
into two kd-plane block-diagonal tiles ([128, 128, 49], off-diagonal
zeros persist from a one-time memset); psum accumulation uses
start=False onto DVE-zeroed banks (skip_group_check).  Bank map
(od + m) keeps the m1 plane-0 banks ones that m0 finished early, so
the PE never stalls at the m boundary.  shell*factor is folded on the
host; DVE fuses psum * shellf during evacuation.
"""

import sys

for _p in ("/opt/trn_rl_repo",):
    if _p not in sys.path:
        sys.path.insert(0, _p)

import numpy as np

NB = 8
KS = 7
S2 = KS * KS               # 49
S3 = KS * KS * KS          # 343
N_CORES = 8
M_PER_CORE = 2             # m-blocks per core
PXS = NB * S3              # 2744: (p, x) free block per m

# Modes (HW-validated rel err):
#   "fp16win": windowed block-diag fp16 (~3e-4)  <- default
#   "bf16win": same scheme in bf16 (~2e-3)
#   "f32r":    full-window block-diag float32r (1.4e-4), prior fallback
MODE = "fp16win"

_CACHE = {}


def _win(k):
    """Valid output range [o0, o1) for kernel offset k: o+k-3 in [0, 7)."""
    return max(0, 3 - k), min(KS, 10 - k)


def _build_nc_win(mode):
    import concourse.tile as tile
    from concourse import bacc, mybir

    f32 = mybir.dt.float32
    f16 = mybir.dt.float16 if mode == "fp16win" else mybir.dt.bfloat16

    nc = bacc.Bacc("TRN2", target_bir_lowering=False, debug=False)

    # host-prearranged inputs (per core):
    #   k1t_d[m, c=(n,j), ((d,h,w), p)]  fp16, transposed + p-innermost
    #   k2w_d[m, kd, n, j, q, t]         fp16, diagonal blocks by kd-plane
    #   shf_d[m, c, p, x]                f32, shell * factor
    k1t_d = nc.dram_tensor("k1t", [M_PER_CORE, 128, S3 * NB], f16,
                           kind="ExternalInput")
    # full pre-zeroed block-diagonal kd-planes: one big contiguous DMA
    # per plane (no SBUF memset, no 16-way diagonal scatter)
    k2w_d = nc.dram_tensor("k2w", [M_PER_CORE, KS, 128, 128 * S2], f16,
                           kind="ExternalInput")
    shf_d = nc.dram_tensor("shf", [M_PER_CORE, 128, NB, S3], f32,
                           kind="ExternalInput")
    out_d = nc.dram_tensor("out", [M_PER_CORE, 128, NB, S3], f32,
                           kind="ExternalOutput")

    with tile.TileContext(nc) as tc:
        with (
            tc.tile_pool(name="persist", bufs=1) as persist,
            tc.tile_pool(name="io", bufs=2) as io,
            tc.tile_pool(name="ps", bufs=1, space="PSUM") as pspool,
        ):
            # two kd-plane weight slots; block-diagonal, zeros persist
            wslots = [
                persist.tile([128, 128, S2], f16, tag=f"w{i}", name=f"w{i}")
                for i in range(2)
            ]
            # 8 psum banks, bank-aligned via full-bank tiles
            banks = [
                pspool.tile([128, 512], f32, tag=f"pb{b}", name=f"pb{b}")
                for b in range(8)
            ]
            bank_mm = [
                b[:, 0:392].rearrange("c (oh ow p) -> c oh ow p", oh=KS, ow=KS)
                for b in banks
            ]
            bank_ev = [
                b[:, 0:392].rearrange("c (oh ow p) -> c p oh ow", oh=KS, ow=KS)
                for b in banks
            ]

            # one-time zeroing of psum banks; m0's first banks (3..6)
            # first so the first matmuls are not held up
            for b in (3, 4, 5, 6, 0, 1, 2, 7):
                nc.vector.memset(banks[b][:, 0:392], 0.0)

            for m in range(M_PER_CORE):
                k1t = io.tile([128, KS, KS, KS, NB], f16, tag="k1t")
                nc.sync.dma_start(
                    out=k1t.rearrange("c a b w p -> c (a b w p)"),
                    in_=k1t_d[m, :, :],
                )

                # slot parity follows the global plane counter so the
                # m1 kd=0 load lands in the slot m0's kd=6 is NOT using
                def load_w(kd, m=m):
                    w = wslots[(m * KS + kd) % 2]
                    nc.sync.dma_start(
                        out=w.rearrange("c a t -> c (a t)"),
                        in_=k2w_d[m, kd, :, :],
                    )

                load_w(0)
                # shf is only needed at evacuation time; keep it behind
                # the critical kd=0 weight plane on the sync queue
                shf = io.tile([128, NB, S3], f32, tag="shf")
                nc.sync.dma_start(out=shf[:, :, :], in_=shf_d[m, :, :, :])
                ost = io.tile([128, NB, S3], f32, tag="ost")
                ost_v = ost.rearrange("c p (od a b) -> c p od a b", od=KS, a=KS)
                shf_v = shf.rearrange("c p (od a b) -> c p od a b", od=KS, a=KS)

                for kd in range(KS):
                    if kd + 1 < KS:
                        load_w(kd + 1)
                    w = wslots[(m * KS + kd) % 2]
                    od0, od1 = _win(kd)
                    for kh in range(KS):
                        oh0, oh1 = _win(kh)
                        for kw in range(KS):
                            ow0, ow1 = _win(kw)
                            lhsT = w[:, :, kh * KS + kw]
                            wlo = ow0 + kw - 3
                            whi = ow1 + kw - 3
                            for od in range(od0, od1):
                                dstb = bank_mm[od + m]
                                d = od + kd - 3
                                for oh in range(oh0, oh1):
                                    nc.tensor.matmul(
                                        dstb[:, oh, ow0:ow1, :],
                                        lhsT,
                                        k1t[:, d, oh + kh - 3, wlo:whi, :],
                                        start=False,
                                        stop=False,
                                        skip_group_check=True,
                                    )

                # evacuate in completion order (od 6,5,4 finish at kd
                # 3,4,5; the rest at kd 6); fuse * shellf.  After each
                # evac, re-zero the bank if the next m needs it (m1 uses
                # banks 1..7; bank 7 is still zero from the initial
                # memset).
                for od in (6, 5, 4, 3, 2, 1, 0):
                    b = od + m
                    nc.vector.tensor_mul(
                        ost_v[:, :, od, :, :],
                        bank_ev[b],
                        shf_v[:, :, od, :, :],
                    )
                    if m + 1 < M_PER_CORE and 1 <= b <= 6:
                        nc.vector.memset(banks[b][:, 0:392], 0.0)

                # out goes via the gpsimd queue: on the sync queue its
                # wait-for-evacs would head-of-line-block the next m's
                # k1t / weight-plane prefetches
                nc.gpsimd.dma_start(out=out_d[m, :, :, :], in_=ost[:, :, :])
    nc.compile()
    return nc


def _build_nc_f32r():
    """Prior fallback: full-window block-diagonal float32r (see git
    history for the original docstring)."""
    import concourse.tile as tile
    from concourse import bacc, mybir

    f32 = mybir.dt.float32
    f32r = mybir.dt.float32r
    WPAD = KS + 1
    SP = KS * KS * WPAD
    DPAD, HPAD, WPAD2 = 13, 13, 14
    PADVOL = DPAD * HPAD * WPAD2

    nc = bacc.Bacc("TRN2", target_bir_lowering=False, debug=False)
    k1 = nc.dram_tensor("k1pad", [M_PER_CORE * NB, 128, PADVOL], f32,
                        kind="ExternalInput")
    k2 = nc.dram_tensor("k2", [M_PER_CORE * NB, 128, S3], f32,
                        kind="ExternalInput")
    shell = nc.dram_tensor("shell", [M_PER_CORE * NB, 128, SP], f32,
                           kind="ExternalInput")
    factor = nc.dram_tensor("factor", [128, 1], f32, kind="ExternalInput")
    zeros = nc.dram_tensor("zeros", [128, 128 * S2], f32, kind="ExternalInput")
    out = nc.dram_tensor("out", [M_PER_CORE * NB, 128, SP], f32,
                         kind="ExternalOutput")

    with tile.TileContext(nc) as tc:
        with (
            tc.tile_pool(name="persist", bufs=1) as persist,
            tc.tile_pool(name="io", bufs=2) as io,
            tc.tile_pool(name="ps", bufs=1, space="PSUM") as pspool,
        ):
            k1t = persist.tile([128, NB, DPAD, HPAD, WPAD2], f32r, tag="k1t")
            wslots = []
            for i in range(2):
                w = persist.tile([128, 128, S2], f32r, tag=f"w{i}",
                                 name=f"w{i}")
                nc.gpsimd.dma_start(out=w.rearrange("c a t -> c (a t)"),
                                    in_=zeros[:, :])
                wslots.append(w)
            fac = persist.tile([128, 1], f32, tag="fac")
            nc.sync.dma_start(out=fac[:, :], in_=factor[:, :])
            psum = [pspool.tile([128, SP], f32, tag=f"pp{p}", name=f"pp{p}")
                    for p in range(NB)]

            for m in range(M_PER_CORE):
                for p in range(NB):
                    nc.gpsimd.dma_start(out=k1t[:, p, :, :, :],
                                        in_=k1[m * NB + p, :, :])
                sh = io.tile([128, NB, SP], f32, tag="shell")
                nc.sync.dma_start(
                    out=sh[:, :, :],
                    in_=shell[m * NB:(m + 1) * NB, :, :].rearrange(
                        "p c s -> c p s"),
                )
                shf = io.tile([128, NB, SP], f32, tag="shellf")
                nc.vector.tensor_scalar_mul(shf[:, :, :], sh[:, :, :],
                                            fac[:, 0:1])
                for kd in range(KS):
                    w = wslots[kd % 2]
                    for n in range(16):
                        nc.gpsimd.dma_start(
                            out=w[n * NB:(n + 1) * NB, n * NB:(n + 1) * NB, :],
                            in_=k2[m * NB:(m + 1) * NB,
                                   n * NB:(n + 1) * NB,
                                   kd * S2:(kd + 1) * S2].rearrange(
                                       "q j t -> j q t"),
                        )
                    for kh in range(KS):
                        for kw in range(KS):
                            t = kh * KS + kw
                            lhsT = w[:, :, t]
                            first = kd == 0 and t == 0
                            last = kd == KS - 1 and t == S2 - 1
                            for p in range(NB):
                                rhs = k1t[:, p, kd:kd + KS, kh:kh + KS,
                                          kw:kw + WPAD]
                                nc.tensor.matmul(psum[p][:, :], lhsT, rhs,
                                                 start=first, stop=last)
                ost = io.tile([128, NB, SP], f32, tag="ost")
                for p in range(NB):
                    nc.vector.tensor_mul(ost[:, p, :], psum[p][:, :],
                                         shf[:, p, :])
                nc.sync.dma_start(
                    out=out[m * NB:(m + 1) * NB, :, :].rearrange(
                        "p c s -> c p s"),
                    in_=ost[:, :, :],
                )
    nc.compile()
    return nc


def _get_nc(mode=None):
    if mode is None:
        mode = MODE
    if mode not in _CACHE:
        if mode in ("fp16win", "bf16win"):
            _CACHE[mode] = _build_nc_win(mode)
        else:
            _CACHE[mode] = _build_nc_f32r()
    return _CACHE[mode]


def _make_in_maps(k1, k2, shell, factor, mode=None):
    if mode is None:
        mode = MODE

    k1 = np.ascontiguousarray(k1.reshape(128, 128, S3), np.float32)
    k2 = np.ascontiguousarray(k2.reshape(128, 128, S3), np.float32)
    shell = shell.reshape(128, 128, S3)
    fval = np.float32(factor.reshape(-1)[0])
    rows = M_PER_CORE * NB

    maps = []
    if mode in ("fp16win", "bf16win"):
        if mode == "bf16win":
            import ml_dtypes
            hdt = ml_dtypes.bfloat16
        else:
            hdt = np.float16
        shellf = (shell * fval).astype(np.float32)
        for c in range(N_CORES):
            sl = slice(c * rows, (c + 1) * rows)
            # k1t[m, c, (x, p)]: rows (m,p) -> free, transposed
            k1c = k1[sl].reshape(M_PER_CORE, NB, 128, S3)
            k1t = np.ascontiguousarray(
                k1c.transpose(0, 2, 3, 1)).astype(hdt).reshape(
                    M_PER_CORE, 128, S3 * NB)
            # k2w[m, kd, (n,j), (n,q), t]: full block-diagonal planes
            # with embedded zeros (single contiguous DMA per plane)
            k2c = k2[sl].reshape(M_PER_CORE, NB, 16, NB, KS, S2)
            blocks = k2c.transpose(0, 4, 2, 3, 1, 5).astype(hdt)  # m,kd,n,j,q,t
            k2w = np.zeros((M_PER_CORE, KS, 16, NB, 16, NB, S2), hdt)
            for n in range(16):
                k2w[:, :, n, :, n] = blocks[:, :, n]
            k2w = k2w.reshape(M_PER_CORE, KS, 128, 128 * S2)
            # shf[m, c, p, x]
            shc = shellf[sl].reshape(M_PER_CORE, NB, 128, S3)
            shf = np.ascontiguousarray(shc.transpose(0, 2, 1, 3))
            maps.append({"k1t": k1t, "k2w": k2w, "shf": shf})
        return maps

    # f32r fallback path
    WPAD = KS + 1
    SP = KS * KS * WPAD
    DPAD, HPAD, WPAD2 = 13, 13, 14
    shell_p = np.zeros((128, 128, KS, KS, WPAD), np.float32)
    shell_p[..., :KS] = shell.reshape(128, 128, KS, KS, KS)
    shell_p = shell_p.reshape(128, 128, SP)
    fac = np.full((128, 1), fval, np.float32)
    k1_pad = np.zeros((128, 128, DPAD, HPAD, WPAD2), np.float32)
    k1_pad[:, :, 3:3 + KS, 3:3 + KS, 3:3 + KS] = k1.reshape(
        128, 128, KS, KS, KS)
    k1_pad = k1_pad.reshape(128, 128, DPAD * HPAD * WPAD2)
    zeros = np.zeros((128, 128 * S2), np.float32)
    for c in range(N_CORES):
        sl = slice(c * rows, (c + 1) * rows)
        maps.append({
            "k1pad": k1_pad[sl], "k2": k2[sl], "shell": shell_p[sl],
            "factor": fac, "zeros": zeros,
        })
    return maps


def _gather(results, mode):
    outs = [np.asarray(r["out"]) for r in results]
    if mode in ("fp16win", "bf16win"):
        # per core: [m, c, p, x] -> rows (2c+m)*8+p
        full = np.empty((128, 128, S3), np.float32)
        for c, o in enumerate(outs):
            full[c * 16:(c + 1) * 16] = o.transpose(0, 2, 1, 3).reshape(
                16, 128, S3)
        return full.reshape(128, 128, KS, KS, KS)
    full = np.concatenate(outs, axis=0)
    WPAD = KS + 1
    full = full.reshape(128, 128, KS, KS, WPAD)[..., :KS]
    return np.ascontiguousarray(full)


def kernel(k1, k2, shell, factor, _trace=False):
    from concourse.bass_utils import run_bass_kernel_spmd

    nc = _get_nc(MODE)
    in_maps = _make_in_maps(
        np.asarray(k1), np.asarray(k2), np.asarray(shell), np.asarray(factor),
        mode=MODE,
    )
    try:
        res = run_bass_kernel_spmd(
            nc, in_maps, core_ids=list(range(N_CORES)), trace=_trace
        )
    except ModuleNotFoundError:
        res = run_bass_kernel_spmd(
            nc, in_maps, core_ids=list(range(N_CORES)), trace=False
        )
    out = _gather(res.results, MODE)
    if _trace:
        return out, res
    return out


# revision 10
# speedup vs baseline: 15.5955x; 1.0200x over previous
"""Trainium2 Bass kernel for nn_ComposedCliffordSteerableKernel.

Computation (see reference): for each of 16x16 (m, n) block pairs, a tiny
3D conv (8,8,7^3) x (8,8,7^3) -> (8,8,7^3) with SAME padding, then
elementwise * shell * factor:

  out[(m,p),(n,q),x] = sum_{j,tap} k1[(m,p),(n,j),x+tap-3] k2[(m,q),(n,j),tap]

Sharding: core c takes output row-blocks m = 2c, 2c+1; no inter-core
communication (gather on host).

Packing ("fp16win", default): per m-block, 128x128 block-diagonal
matmuls -- contraction partitions (n,j) = 16 pairs x 8 input blades,
output partitions (n,q).  The pair index n must live in the contraction
partitions (the rhs is shared by all output columns), which caps useful
MACs at 16*8*8 = 1024 per streamed PSUM row; the optimum is therefore to
stream ONLY valid rows.  All three window dims of the conv are clipped:

  - od: PSUM is split per output depth -- bank (od+m)%8 holds the
    (oh, ow, p) = 7*7*8 = 392-float slab for that od; a tap (kd,*,*)
    only touches banks whose od has od+kd-3 in [0,7).
  - oh: one matmul per (tap, od, oh) with oh restricted to its valid
    window; dst = bank[:, oh, ow0:ow1, :] stays a contiguous run.
  - ow: the innermost (ow, p) run is clipped to the valid ow window of
    kw; rhs = k1t[:, d, h, wlo:whi, :] is the matching contiguous run
    (k1t is held un-padded -- every read lands in the 7^3 interior).

Streamed rows/core = 2m * 8p * 37^3 = 810,448 -- every row is a fully
valid output contribution (37 = sum_k (7-|k-3|) per dim).  fp16 keeps
1 cycle/row on the PE and ~3e-4 rel err.  Weights are DMA-scattered
into two kd-plane block-diagonal tiles ([128, 128, 49], off-diagonal
zeros persist from a one-time memset); psum accumulation uses
start=False onto DVE-zeroed banks (skip_group_check).  Bank map
(od + m) keeps the m1 plane-0 banks ones that m0 finished early, so
the PE never stalls at the m boundary.  shell*factor is folded on the
host; DVE fuses psum * shellf during evacuation.
"""

import sys

for _p in ("/opt/trn_rl_repo",):
    if _p not in sys.path:
        sys.path.insert(0, _p)

import numpy as np

NB = 8
KS = 7
S2 = KS * KS               # 49
S3 = KS * KS * KS          # 343
N_CORES = 8
M_PER_CORE = 2             # m-blocks per core
PXS = NB * S3              # 2744: (p, x) free block per m

# Modes (HW-validated rel err):
#   "fp16win": windowed block-diag fp16 (~3e-4)  <- default
#   "bf16win": same scheme in bf16 (~2e-3)
#   "f32r":    full-window block-diag float32r (1.4e-4), prior fallback
MODE = "fp16win"

_CACHE = {}


def _win(k):
    """Valid output range [o0, o1) for kernel offset k: o+k-3 in [0, 7)."""
    return max(0, 3 - k), min(KS, 10 - k)


def _build_nc_win(mode):
    import concourse.tile as tile
    from concourse import bacc, mybir

    f32 = mybir.dt.float32
    f16 = mybir.dt.float16 if mode == "fp16win" else mybir.dt.bfloat16

    nc = bacc.Bacc("TRN2", target_bir_lowering=False, debug=False)

    # host-prearranged inputs (per core):
    #   k1t_d[m, c=(n,j), ((d,h,w), p)]  fp16, transposed + p-innermost
    #   k2w_d[m, kd, n, j, q, t]         fp16, diagonal blocks by kd-plane
    #   shf_d[m, c, p, x]                f32, shell * factor
    k1t_d = nc.dram_tensor("k1t", [M_PER_CORE, 128, S3 * NB], f16,
                           kind="ExternalInput")
    # full pre-zeroed block-diagonal kd-planes: one big contiguous DMA
    # per plane (no SBUF memset, no 16-way diagonal scatter)
    k2w_d = nc.dram_tensor("k2w", [M_PER_CORE, KS, 128, 128 * S2], f16,
                           kind="ExternalInput")
    shf_d = nc.dram_tensor("shf", [M_PER_CORE, 128, NB, S3], f32,
                           kind="ExternalInput")
    out_d = nc.dram_tensor("out", [M_PER_CORE, 128, NB, S3], f32,
                           kind="ExternalOutput")

    with tile.TileContext(nc) as tc:
        with (
            tc.tile_pool(name="persist", bufs=1) as persist,
            tc.tile_pool(name="io", bufs=2) as io,
            tc.tile_pool(name="ps", bufs=1, space="PSUM") as pspool,
        ):
            # two kd-plane weight slots; block-diagonal, zeros persist
            # t-major weight layout: lhsT = w[:, t, :]; t-chunk DMA
            # slices stay contiguous
            wslots = [
                persist.tile([128, S2, 128], f16, tag=f"w{i}", name=f"w{i}")
                for i in range(2)
            ]
            # 8 psum banks, bank-aligned via full-bank tiles
            banks = [
                pspool.tile([128, 512], f32, tag=f"pb{b}", name=f"pb{b}")
                for b in range(8)
            ]
            bank_mm = [
                b[:, 0:392].rearrange("c (oh ow p) -> c oh ow p", oh=KS, ow=KS)
                for b in banks
            ]
            bank_ev = [
                b[:, 0:392].rearrange("c (oh ow p) -> c p oh ow", oh=KS, ow=KS)
                for b in banks
            ]

            # one-time zeroing of psum banks; m0's first banks (3..6)
            # first so the first matmuls are not held up
            for b in (3, 4, 5, 6, 0, 1, 2, 7):
                nc.vector.memset(banks[b][:, 0:392], 0.0)

            for m in range(M_PER_CORE):
                # slot parity follows the global plane counter so the
                # m1 kd=0 load lands in the slot m0's kd=6 is NOT using
                def load_w(kd, m=m):
                    w = wslots[(m * KS + kd) % 2]
                    nc.sync.dma_start(
                        out=w.rearrange("c t a -> c (t a)"),
                        in_=k2w_d[m, kd, :, :],
                    )

                k1t = io.tile([128, KS, KS, KS, NB], f16, tag="k1t")
                if m == 0:
                    # split the very first plane into t-chunks so the
                    # first taps start after ~1/4 of it has landed (the
                    # DMA transfers serialize on the shared DMA engines)
                    w0 = wslots[0]
                    nc.sync.dma_start(
                        out=w0[:, 0:13, :].rearrange("c t a -> c (t a)"),
                        in_=k2w_d[0, 0, :, 0:128 * 13],
                    )
                    nc.sync.dma_start(
                        out=k1t.rearrange("c a b w p -> c (a b w p)"),
                        in_=k1t_d[m, :, :],
                    )
                    for t0, t1 in ((13, 25), (25, 37), (37, S2)):
                        nc.sync.dma_start(
                            out=w0[:, t0:t1, :].rearrange("c t a -> c (t a)"),
                            in_=k2w_d[0, 0, :, 128 * t0:128 * t1],
                        )
                else:
                    nc.sync.dma_start(
                        out=k1t.rearrange("c a b w p -> c (a b w p)"),
                        in_=k1t_d[m, :, :],
                    )
                    load_w(0)
                # shf is only needed at evacuation time; keep it behind
                # the critical kd=0 weight plane on the sync queue
                shf = io.tile([128, NB, S3], f32, tag="shf")
                nc.sync.dma_start(out=shf[:, :, :], in_=shf_d[m, :, :, :])
                ost = io.tile([128, NB, S3], f32, tag="ost")
                ost_v = ost.rearrange("c p (od a b) -> c p od a b", od=KS, a=KS)
                shf_v = shf.rearrange("c p (od a b) -> c p od a b", od=KS, a=KS)

                def emit_tap(kd, kh, kw, od, w):
                    oh0, oh1 = _win(kh)
                    ow0, ow1 = _win(kw)
                    lhsT = w[:, kh * KS + kw, :]
                    wlo = ow0 + kw - 3
                    whi = ow1 + kw - 3
                    dstb = bank_mm[od + m]
                    d = od + kd - 3
                    for oh in range(oh0, oh1):
                        nc.tensor.matmul(
                            dstb[:, oh, ow0:ow1, :],
                            lhsT,
                            k1t[:, d, oh + kh - 3, wlo:whi, :],
                            start=False,
                            stop=False,
                            skip_group_check=True,
                        )

                for kd in range(KS):
                    if kd + 1 < KS:
                        load_w(kd + 1)
                    w = wslots[(m * KS + kd) % 2]
                    od0, od1 = _win(kd)
                    if kd < KS - 1:
                        for kh in range(KS):
                            for kw in range(KS):
                                for od in range(od0, od1):
                                    emit_tap(kd, kh, kw, od, w)
                    else:
                        # last plane od-outer, descending: banks then
                        # complete in evacuation order (od 3 first) so
                        # the tail is one evac + one out-chunk
                        for od in range(od1 - 1, od0 - 1, -1):
                            for kh in range(KS):
                                for kw in range(KS):
                                    emit_tap(kd, kh, kw, od, w)

                # evacuate in completion order (od 6,5,4 finish at kd
                # 3,4,5; od 3..0 in that order inside the kd=6 plane);
                # fuse * shellf, stream each od chunk out immediately
                # (gpsimd queue: the sync queue would head-of-line-block
                # the next m's prefetches), and re-zero the bank if the
                # next m needs it (m1 uses banks 1..7; bank 7 is still
                # zero from the initial memset).
                for od in (6, 5, 4, 3, 2, 1, 0):
                    b = od + m
                    nc.vector.tensor_mul(
                        ost_v[:, :, od, :, :],
                        bank_ev[b],
                        shf_v[:, :, od, :, :],
                    )
                    nc.gpsimd.dma_start(
                        out=out_d[m, :, :, od * S2:(od + 1) * S2],
                        in_=ost[:, :, od * S2:(od + 1) * S2],
                    )
                    if m + 1 < M_PER_CORE and 1 <= b <= 6:
                        nc.vector.memset(banks[b][:, 0:392], 0.0)
    nc.compile()
    return nc


def _build_nc_f32r():
    """Prior fallback: full-window block-diagonal float32r (see git
    history for the original docstring)."""
    import concourse.tile as tile
    from concourse import bacc, mybir

    f32 = mybir.dt.float32
    f32r = mybir.dt.float32r
    WPAD = KS + 1
    SP = KS * KS * WPAD
    DPAD, HPAD, WPAD2 = 13, 13, 14
    PADVOL = DPAD * HPAD * WPAD2

    nc = bacc.Bacc("TRN2", target_bir_lowering=False, debug=False)
    k1 = nc.dram_tensor("k1pad", [M_PER_CORE * NB, 128, PADVOL], f32,
                        kind="ExternalInput")
    k2 = nc.dram_tensor("k2", [M_PER_CORE * NB, 128, S3], f32,
                        kind="ExternalInput")
    shell = nc.dram_tensor("shell", [M_PER_CORE * NB, 128, SP], f32,
                           kind="ExternalInput")
    factor = nc.dram_tensor("factor", [128, 1], f32, kind="ExternalInput")
    zeros = nc.dram_tensor("zeros", [128, 128 * S2], f32, kind="ExternalInput")
    out = nc.dram_tensor("out", [M_PER_CORE * NB, 128, SP], f32,
                         kind="ExternalOutput")

    with tile.TileContext(nc) as tc:
        with (
            tc.tile_pool(name="persist", bufs=1) as persist,
            tc.tile_pool(name="io", bufs=2) as io,
            tc.tile_pool(name="ps", bufs=1, space="PSUM") as pspool,
        ):
            k1t = persist.tile([128, NB, DPAD, HPAD, WPAD2], f32r, tag="k1t")
            wslots = []
            for i in range(2):
                w = persist.tile([128, 128, S2], f32r, tag=f"w{i}",
                                 name=f"w{i}")
                nc.gpsimd.dma_start(out=w.rearrange("c a t -> c (a t)"),
                                    in_=zeros[:, :])
                wslots.append(w)
            fac = persist.tile([128, 1], f32, tag="fac")
            nc.sync.dma_start(out=fac[:, :], in_=factor[:, :])
            psum = [pspool.tile([128, SP], f32, tag=f"pp{p}", name=f"pp{p}")
                    for p in range(NB)]

            for m in range(M_PER_CORE):
                for p in range(NB):
                    nc.gpsimd.dma_start(out=k1t[:, p, :, :, :],
                                        in_=k1[m * NB + p, :, :])
                sh = io.tile([128, NB, SP], f32, tag="shell")
                nc.sync.dma_start(
                    out=sh[:, :, :],
                    in_=shell[m * NB:(m + 1) * NB, :, :].rearrange(
                        "p c s -> c p s"),
                )
                shf = io.tile([128, NB, SP], f32, tag="shellf")
                nc.vector.tensor_scalar_mul(shf[:, :, :], sh[:, :, :],
                                            fac[:, 0:1])
                for kd in range(KS):
                    w = wslots[kd % 2]
                    for n in range(16):
                        nc.gpsimd.dma_start(
                            out=w[n * NB:(n + 1) * NB, n * NB:(n + 1) * NB, :],
                            in_=k2[m * NB:(m + 1) * NB,
                                   n * NB:(n + 1) * NB,
                                   kd * S2:(kd + 1) * S2].rearrange(
                                       "q j t -> j q t"),
                        )
                    for kh in range(KS):
                        for kw in range(KS):
                            t = kh * KS + kw
                            lhsT = w[:, :, t]
                            first = kd == 0 and t == 0
                            last = kd == KS - 1 and t == S2 - 1
                            for p in range(NB):
                                rhs = k1t[:, p, kd:kd + KS, kh:kh + KS,
                                          kw:kw + WPAD]
                                nc.tensor.matmul(psum[p][:, :], lhsT, rhs,
                                                 start=first, stop=last)
                ost = io.tile([128, NB, SP], f32, tag="ost")
                for p in range(NB):
                    nc.vector.tensor_mul(ost[:, p, :], psum[p][:, :],
                                         shf[:, p, :])
                nc.sync.dma_start(
                    out=out[m * NB:(m + 1) * NB, :, :].rearrange(
                        "p c s -> c p s"),
                    in_=ost[:, :, :],
                )
    nc.compile()
    return nc


def _get_nc(mode=None):
    if mode is None:
        mode = MODE
    if mode not in _CACHE:
        if mode in ("fp16win", "bf16win"):
            _CACHE[mode] = _build_nc_win(mode)
        else:
            _CACHE[mode] = _build_nc_f32r()
    return _CACHE[mode]


def _make_in_maps(k1, k2, shell, factor, mode=None):
    if mode is None:
        mode = MODE

    k1 = np.ascontiguousarray(k1.reshape(128, 128, S3), np.float32)
    k2 = np.ascontiguousarray(k2.reshape(128, 128, S3), np.float32)
    shell = shell.reshape(128, 128, S3)
    fval = np.float32(factor.reshape(-1)[0])
    rows = M_PER_CORE * NB

    maps = []
    if mode in ("fp16win", "bf16win"):
        if mode == "bf16win":
            import ml_dtypes
            hdt = ml_dtypes.bfloat16
        else:
            hdt = np.float16
        shellf = (shell * fval).astype(np.float32)
        for c in range(N_CORES):
            sl = slice(c * rows, (c + 1) * rows)
            # k1t[m, c, (x, p)]: rows (m,p) -> free, transposed
            k1c = k1[sl].reshape(M_PER_CORE, NB, 128, S3)
            k1t = np.ascontiguousarray(
                k1c.transpose(0, 2, 3, 1)).astype(hdt).reshape(
                    M_PER_CORE, 128, S3 * NB)
            # k2w[m, kd, (n,j), (n,q), t]: full block-diagonal planes
            # with embedded zeros (single contiguous DMA per plane)
            k2c = k2[sl].reshape(M_PER_CORE, NB, 16, NB, KS, S2)
            blocks = k2c.transpose(0, 4, 2, 3, 1, 5).astype(hdt)  # m,kd,n,j,q,t
            k2w = np.zeros((M_PER_CORE, KS, 16, NB, S2, 16, NB), hdt)
            for n in range(16):
                k2w[:, :, n, :, :, n] = blocks[:, :, n].transpose(0, 1, 2, 4, 3)
            k2w = k2w.reshape(M_PER_CORE, KS, 128, 128 * S2)
            # shf[m, c, p, x]
            shc = shellf[sl].reshape(M_PER_CORE, NB, 128, S3)
            shf = np.ascontiguousarray(shc.transpose(0, 2, 1, 3))
            maps.append({"k1t": k1t, "k2w": k2w, "shf": shf})
        return maps

    # f32r fallback path
    WPAD = KS + 1
    SP = KS * KS * WPAD
    DPAD, HPAD, WPAD2 = 13, 13, 14
    shell_p = np.zeros((128, 128, KS, KS, WPAD), np.float32)
    shell_p[..., :KS] = shell.reshape(128, 128, KS, KS, KS)
    shell_p = shell_p.reshape(128, 128, SP)
    fac = np.full((128, 1), fval, np.float32)
    k1_pad = np.zeros((128, 128, DPAD, HPAD, WPAD2), np.float32)
    k1_pad[:, :, 3:3 + KS, 3:3 + KS, 3:3 + KS] = k1.reshape(
        128, 128, KS, KS, KS)
    k1_pad = k1_pad.reshape(128, 128, DPAD * HPAD * WPAD2)
    zeros = np.zeros((128, 128 * S2), np.float32)
    for c in range(N_CORES):
        sl = slice(c * rows, (c + 1) * rows)
        maps.append({
            "k1pad": k1_pad[sl], "k2": k2[sl], "shell": shell_p[sl],
            "factor": fac, "zeros": zeros,
        })
    return maps


def _gather(results, mode):
    outs = [np.asarray(r["out"]) for r in results]
    if mode in ("fp16win", "bf16win"):
        # per core: [m, c, p, x] -> rows (2c+m)*8+p
        full = np.empty((128, 128, S3), np.float32)
        for c, o in enumerate(outs):
            full[c * 16:(c + 1) * 16] = o.transpose(0, 2, 1, 3).reshape(
                16, 128, S3)
        return full.reshape(128, 128, KS, KS, KS)
    full = np.concatenate(outs, axis=0)
    WPAD = KS + 1
    full = full.reshape(128, 128, KS, KS, WPAD)[..., :KS]
    return np.ascontiguousarray(full)


def kernel(k1, k2, shell, factor, _trace=False):
    from concourse.bass_utils import run_bass_kernel_spmd

    nc = _get_nc(MODE)
    in_maps = _make_in_maps(
        np.asarray(k1), np.asarray(k2), np.asarray(shell), np.asarray(factor),
        mode=MODE,
    )
    try:
        res = run_bass_kernel_spmd(
            nc, in_maps, core_ids=list(range(N_CORES)), trace=_trace
        )
    except ModuleNotFoundError:
        res = run_bass_kernel_spmd(
            nc, in_maps, core_ids=list(range(N_CORES)), trace=False
        )
    out = _gather(res.results, MODE)
    if _trace:
        return out, res
    return out


# revision 11
# speedup vs baseline: 15.7218x; 1.0081x over previous
"""Trainium2 Bass kernel for nn_ComposedCliffordSteerableKernel.

Computation (see reference): for each of 16x16 (m, n) block pairs, a tiny
3D conv (8,8,7^3) x (8,8,7^3) -> (8,8,7^3) with SAME padding, then
elementwise * shell * factor:

  out[(m,p),(n,q),x] = sum_{j,tap} k1[(m,p),(n,j),x+tap-3] k2[(m,q),(n,j),tap]

Sharding: core c takes output row-blocks m = 2c, 2c+1; no inter-core
communication (gather on host).

Packing ("fp16win", default): per m-block, 128x128 block-diagonal
matmuls -- contraction partitions (n,j) = 16 pairs x 8 input blades,
output partitions (n,q).  The pair index n must live in the contraction
partitions (the rhs is shared by all output columns), which caps useful
MACs at 16*8*8 = 1024 per streamed PSUM row; the optimum is therefore to
stream ONLY valid rows.  All three window dims of the conv are clipped:

  - od: PSUM is split per output depth -- bank (od+m)%8 holds the
    (oh, ow, p) = 7*7*8 = 392-float slab for that od; a tap (kd,*,*)
    only touches banks whose od has od+kd-3 in [0,7).
  - oh: one matmul per (tap, od, oh) with oh restricted to its valid
    window; dst = bank[:, oh, ow0:ow1, :] stays a contiguous run.
  - ow: the innermost (ow, p) run is clipped to the valid ow window of
    kw; rhs = k1t[:, d, h, wlo:whi, :] is the matching contiguous run
    (k1t is held un-padded -- every read lands in the 7^3 interior).

Streamed rows/core = 2m * 8p * 37^3 = 810,448 -- every row is a fully
valid output contribution (37 = sum_k (7-|k-3|) per dim).  fp16 keeps
1 cycle/row on the PE and ~3e-4 rel err.  Weights are DMA-scattered
into two kd-plane block-diagonal tiles ([128, 128, 49], off-diagonal
zeros persist from a one-time memset); psum accumulation uses
start=False onto DVE-zeroed banks (skip_group_check).  Bank map
(od + m) keeps the m1 plane-0 banks ones that m0 finished early, so
the PE never stalls at the m boundary.  shell*factor is folded on the
host; DVE fuses psum * shellf during evacuation.
"""

import sys

for _p in ("/opt/trn_rl_repo",):
    if _p not in sys.path:
        sys.path.insert(0, _p)

import numpy as np

NB = 8
KS = 7
S2 = KS * KS               # 49
S3 = KS * KS * KS          # 343
N_CORES = 8
M_PER_CORE = 2             # m-blocks per core
PXS = NB * S3              # 2744: (p, x) free block per m

# Modes (HW-validated rel err):
#   "fp16win": windowed block-diag fp16 (~3e-4)  <- default
#   "bf16win": same scheme in bf16 (~2e-3)
#   "f32r":    full-window block-diag float32r (1.4e-4), prior fallback
MODE = "fp16win"

_CACHE = {}


def _win(k):
    """Valid output range [o0, o1) for kernel offset k: o+k-3 in [0, 7)."""
    return max(0, 3 - k), min(KS, 10 - k)


def _build_nc_win(mode):
    import concourse.tile as tile
    from concourse import bacc, mybir

    f32 = mybir.dt.float32
    f16 = mybir.dt.float16 if mode == "fp16win" else mybir.dt.bfloat16

    nc = bacc.Bacc("TRN2", target_bir_lowering=False, debug=False)

    # host-prearranged inputs (per core):
    #   k1t_d[m, c=(n,j), ((d,h,w), p)]  fp16, transposed + p-innermost
    #   k2w_d[m, kd, n, j, q, t]         fp16, diagonal blocks by kd-plane
    #   shf_d[m, c, p, x]                f32, shell * factor
    k1t_d = nc.dram_tensor("k1t", [M_PER_CORE, 128, S3 * NB], f16,
                           kind="ExternalInput")
    # full pre-zeroed block-diagonal kd-planes: one big contiguous DMA
    # per plane (no SBUF memset, no 16-way diagonal scatter)
    k2w_d = nc.dram_tensor("k2w", [M_PER_CORE, KS, 128, 128 * S2], f16,
                           kind="ExternalInput")
    shf_d = nc.dram_tensor("shf", [M_PER_CORE, 128, NB, S3], f32,
                           kind="ExternalInput")
    # out is chunked per od; [m, c, od, p, (oh,ow)] keeps each
    # chunk's per-partition run at 8*49*4B (no sub-512B RMW penalty)
    out_d = nc.dram_tensor("out", [M_PER_CORE, 128, KS, NB, S2], f32,
                           kind="ExternalOutput")

    with tile.TileContext(nc) as tc:
        with (
            tc.tile_pool(name="persist", bufs=1) as persist,
            tc.tile_pool(name="io", bufs=2) as io,
            tc.tile_pool(name="ps", bufs=1, space="PSUM") as pspool,
        ):
            # two kd-plane weight slots; block-diagonal, zeros persist
            # t-major weight layout: lhsT = w[:, t, :]; t-chunk DMA
            # slices stay contiguous
            wslots = [
                persist.tile([128, S2, 128], f16, tag=f"w{i}", name=f"w{i}")
                for i in range(2)
            ]
            # 8 psum banks, bank-aligned via full-bank tiles
            banks = [
                pspool.tile([128, 512], f32, tag=f"pb{b}", name=f"pb{b}")
                for b in range(8)
            ]
            bank_mm = [
                b[:, 0:392].rearrange("c (oh ow p) -> c oh ow p", oh=KS, ow=KS)
                for b in banks
            ]
            bank_ev = [
                b[:, 0:392].rearrange("c (oh ow p) -> c p oh ow", oh=KS, ow=KS)
                for b in banks
            ]

            # one-time zeroing of psum banks; m0's first banks (3..6)
            # first so the first matmuls are not held up
            for b in (3, 4, 5, 6, 0, 1, 2, 7):
                nc.vector.memset(banks[b][:, 0:392], 0.0)

            for m in range(M_PER_CORE):
                # slot parity follows the global plane counter so the
                # m1 kd=0 load lands in the slot m0's kd=6 is NOT using
                def load_w(kd, m=m):
                    w = wslots[(m * KS + kd) % 2]
                    nc.sync.dma_start(
                        out=w.rearrange("c t a -> c (t a)"),
                        in_=k2w_d[m, kd, :, :],
                    )

                k1t = io.tile([128, KS, KS, KS, NB], f16, tag="k1t")
                if m == 0:
                    # startup criticals in small pieces: first w-plane in
                    # t-chunks, k1t split by d (the kd=0 plane only ever
                    # reads d 0..3) -- the DMA transfers serialize on the
                    # shared DMA engines, so the first taps can start
                    # after only ~2.2us of transfer
                    w0 = wslots[0]
                    nc.sync.dma_start(
                        out=w0[:, 0:7, :].rearrange("c t a -> c (t a)"),
                        in_=k2w_d[0, 0, :, 0:128 * 7],
                    )
                    nc.sync.dma_start(
                        out=k1t[:, 0:4, :, :, :].rearrange(
                            "c a b w p -> c (a b w p)"),
                        in_=k1t_d[m, :, 0:4 * S2 * NB],
                    )
                    for t0, t1 in ((7, 21), (21, 35), (35, S2)):
                        nc.sync.dma_start(
                            out=w0[:, t0:t1, :].rearrange("c t a -> c (t a)"),
                            in_=k2w_d[0, 0, :, 128 * t0:128 * t1],
                        )
                    nc.sync.dma_start(
                        out=k1t[:, 4:KS, :, :, :].rearrange(
                            "c a b w p -> c (a b w p)"),
                        in_=k1t_d[m, :, 4 * S2 * NB:],
                    )
                else:
                    nc.sync.dma_start(
                        out=k1t.rearrange("c a b w p -> c (a b w p)"),
                        in_=k1t_d[m, :, :],
                    )
                    load_w(0)
                # shf is only needed at evacuation time; keep it behind
                # the critical kd=0 weight plane on the sync queue
                shf = io.tile([128, NB, S3], f32, tag="shf")
                nc.sync.dma_start(out=shf[:, :, :], in_=shf_d[m, :, :, :])
                ost = io.tile([128, KS, NB, S2], f32, tag="ost")
                shf_v = shf.rearrange("c p (od a b) -> c od p (a b)", od=KS, a=KS)

                def emit_tap(kd, kh, kw, od, w):
                    oh0, oh1 = _win(kh)
                    ow0, ow1 = _win(kw)
                    lhsT = w[:, kh * KS + kw, :]
                    wlo = ow0 + kw - 3
                    whi = ow1 + kw - 3
                    dstb = bank_mm[od + m]
                    d = od + kd - 3
                    for oh in range(oh0, oh1):
                        nc.tensor.matmul(
                            dstb[:, oh, ow0:ow1, :],
                            lhsT,
                            k1t[:, d, oh + kh - 3, wlo:whi, :],
                            start=False,
                            stop=False,
                            skip_group_check=True,
                        )

                for kd in range(KS):
                    if kd + 1 < KS:
                        load_w(kd + 1)
                    w = wslots[(m * KS + kd) % 2]
                    od0, od1 = _win(kd)
                    if kd < KS - 1:
                        for kh in range(KS):
                            for kw in range(KS):
                                for od in range(od0, od1):
                                    emit_tap(kd, kh, kw, od, w)
                    else:
                        # last plane od-outer, descending: banks then
                        # complete in evacuation order (od 3 first) so
                        # the tail is one evac + one out-chunk
                        for od in range(od1 - 1, od0 - 1, -1):
                            for kh in range(KS):
                                for kw in range(KS):
                                    emit_tap(kd, kh, kw, od, w)

                # evacuate in completion order (od 6,5,4 finish at kd
                # 3,4,5; od 3..0 in that order inside the kd=6 plane);
                # fuse * shellf, stream each od chunk out immediately
                # (gpsimd queue: the sync queue would head-of-line-block
                # the next m's prefetches), and re-zero the bank if the
                # next m needs it (m1 uses banks 1..7; bank 7 is still
                # zero from the initial memset).
                out_q = nc.gpsimd if m + 1 < M_PER_CORE else nc.sync
                for od in (6, 5, 4, 3, 2, 1, 0):
                    b = od + m
                    nc.vector.tensor_mul(
                        ost[:, od, :, :],
                        bank_ev[b],
                        shf_v[:, od, :, :],
                    )
                    out_q.dma_start(
                        out=out_d[m, :, od, :, :],
                        in_=ost[:, od, :, :],
                    )
                    if m + 1 < M_PER_CORE and 1 <= b <= 6:
                        nc.vector.memset(banks[b][:, 0:392], 0.0)
    nc.compile()
    return nc


def _build_nc_f32r():
    """Prior fallback: full-window block-diagonal float32r (see git
    history for the original docstring)."""
    import concourse.tile as tile
    from concourse import bacc, mybir

    f32 = mybir.dt.float32
    f32r = mybir.dt.float32r
    WPAD = KS + 1
    SP = KS * KS * WPAD
    DPAD, HPAD, WPAD2 = 13, 13, 14
    PADVOL = DPAD * HPAD * WPAD2

    nc = bacc.Bacc("TRN2", target_bir_lowering=False, debug=False)
    k1 = nc.dram_tensor("k1pad", [M_PER_CORE * NB, 128, PADVOL], f32,
                        kind="ExternalInput")
    k2 = nc.dram_tensor("k2", [M_PER_CORE * NB, 128, S3], f32,
                        kind="ExternalInput")
    shell = nc.dram_tensor("shell", [M_PER_CORE * NB, 128, SP], f32,
                           kind="ExternalInput")
    factor = nc.dram_tensor("factor", [128, 1], f32, kind="ExternalInput")
    zeros = nc.dram_tensor("zeros", [128, 128 * S2], f32, kind="ExternalInput")
    out = nc.dram_tensor("out", [M_PER_CORE * NB, 128, SP], f32,
                         kind="ExternalOutput")

    with tile.TileContext(nc) as tc:
        with (
            tc.tile_pool(name="persist", bufs=1) as persist,
            tc.tile_pool(name="io", bufs=2) as io,
            tc.tile_pool(name="ps", bufs=1, space="PSUM") as pspool,
        ):
            k1t = persist.tile([128, NB, DPAD, HPAD, WPAD2], f32r, tag="k1t")
            wslots = []
            for i in range(2):
                w = persist.tile([128, 128, S2], f32r, tag=f"w{i}",
                                 name=f"w{i}")
                nc.gpsimd.dma_start(out=w.rearrange("c a t -> c (a t)"),
                                    in_=zeros[:, :])
                wslots.append(w)
            fac = persist.tile([128, 1], f32, tag="fac")
            nc.sync.dma_start(out=fac[:, :], in_=factor[:, :])
            psum = [pspool.tile([128, SP], f32, tag=f"pp{p}", name=f"pp{p}")
                    for p in range(NB)]

            for m in range(M_PER_CORE):
                for p in range(NB):
                    nc.gpsimd.dma_start(out=k1t[:, p, :, :, :],
                                        in_=k1[m * NB + p, :, :])
                sh = io.tile([128, NB, SP], f32, tag="shell")
                nc.sync.dma_start(
                    out=sh[:, :, :],
                    in_=shell[m * NB:(m + 1) * NB, :, :].rearrange(
                        "p c s -> c p s"),
                )
                shf = io.tile([128, NB, SP], f32, tag="shellf")
                nc.vector.tensor_scalar_mul(shf[:, :, :], sh[:, :, :],
                                            fac[:, 0:1])
                for kd in range(KS):
                    w = wslots[kd % 2]
                    for n in range(16):
                        nc.gpsimd.dma_start(
                            out=w[n * NB:(n + 1) * NB, n * NB:(n + 1) * NB, :],
                            in_=k2[m * NB:(m + 1) * NB,
                                   n * NB:(n + 1) * NB,
                                   kd * S2:(kd + 1) * S2].rearrange(
                                       "q j t -> j q t"),
                        )
                    for kh in range(KS):
                        for kw in range(KS):
                            t = kh * KS + kw
                            lhsT = w[:, :, t]
                            first = kd == 0 and t == 0
                            last = kd == KS - 1 and t == S2 - 1
                            for p in range(NB):
                                rhs = k1t[:, p, kd:kd + KS, kh:kh + KS,
                                          kw:kw + WPAD]
                                nc.tensor.matmul(psum[p][:, :], lhsT, rhs,
                                                 start=first, stop=last)
                ost = io.tile([128, NB, SP], f32, tag="ost")
                for p in range(NB):
                    nc.vector.tensor_mul(ost[:, p, :], psum[p][:, :],
                                         shf[:, p, :])
                nc.sync.dma_start(
                    out=out[m * NB:(m + 1) * NB, :, :].rearrange(
                        "p c s -> c p s"),
                    in_=ost[:, :, :],
                )
    nc.compile()
    return nc


def _get_nc(mode=None):
    if mode is None:
        mode = MODE
    if mode not in _CACHE:
        if mode in ("fp16win", "bf16win"):
            _CACHE[mode] = _build_nc_win(mode)
        else:
            _CACHE[mode] = _build_nc_f32r()
    return _CACHE[mode]


def _make_in_maps(k1, k2, shell, factor, mode=None):
    if mode is None:
        mode = MODE

    k1 = np.ascontiguousarray(k1.reshape(128, 128, S3), np.float32)
    k2 = np.ascontiguousarray(k2.reshape(128, 128, S3), np.float32)
    shell = shell.reshape(128, 128, S3)
    fval = np.float32(factor.reshape(-1)[0])
    rows = M_PER_CORE * NB

    maps = []
    if mode in ("fp16win", "bf16win"):
        if mode == "bf16win":
            import ml_dtypes
            hdt = ml_dtypes.bfloat16
        else:
            hdt = np.float16
        shellf = (shell * fval).astype(np.float32)
        for c in range(N_CORES):
            sl = slice(c * rows, (c + 1) * rows)
            # k1t[m, c, (x, p)]: rows (m,p) -> free, transposed
            k1c = k1[sl].reshape(M_PER_CORE, NB, 128, S3)
            k1t = np.ascontiguousarray(
                k1c.transpose(0, 2, 3, 1)).astype(hdt).reshape(
                    M_PER_CORE, 128, S3 * NB)
            # k2w[m, kd, (n,j), (n,q), t]: full block-diagonal planes
            # with embedded zeros (single contiguous DMA per plane)
            k2c = k2[sl].reshape(M_PER_CORE, NB, 16, NB, KS, S2)
            blocks = k2c.transpose(0, 4, 2, 3, 1, 5).astype(hdt)  # m,kd,n,j,q,t
            k2w = np.zeros((M_PER_CORE, KS, 16, NB, S2, 16, NB), hdt)
            for n in range(16):
                k2w[:, :, n, :, :, n] = blocks[:, :, n].transpose(0, 1, 2, 4, 3)
            k2w = k2w.reshape(M_PER_CORE, KS, 128, 128 * S2)
            # shf[m, c, p, x]
            shc = shellf[sl].reshape(M_PER_CORE, NB, 128, S3)
            shf = np.ascontiguousarray(shc.transpose(0, 2, 1, 3))
            maps.append({"k1t": k1t, "k2w": k2w, "shf": shf})
        return maps

    # f32r fallback path
    WPAD = KS + 1
    SP = KS * KS * WPAD
    DPAD, HPAD, WPAD2 = 13, 13, 14
    shell_p = np.zeros((128, 128, KS, KS, WPAD), np.float32)
    shell_p[..., :KS] = shell.reshape(128, 128, KS, KS, KS)
    shell_p = shell_p.reshape(128, 128, SP)
    fac = np.full((128, 1), fval, np.float32)
    k1_pad = np.zeros((128, 128, DPAD, HPAD, WPAD2), np.float32)
    k1_pad[:, :, 3:3 + KS, 3:3 + KS, 3:3 + KS] = k1.reshape(
        128, 128, KS, KS, KS)
    k1_pad = k1_pad.reshape(128, 128, DPAD * HPAD * WPAD2)
    zeros = np.zeros((128, 128 * S2), np.float32)
    for c in range(N_CORES):
        sl = slice(c * rows, (c + 1) * rows)
        maps.append({
            "k1pad": k1_pad[sl], "k2": k2[sl], "shell": shell_p[sl],
            "factor": fac, "zeros": zeros,
        })
    return maps


def _gather(results, mode):
    outs = [np.asarray(r["out"]) for r in results]
    if mode in ("fp16win", "bf16win"):
        # per core: [m, c, od, p, (oh,ow)] -> rows (2c+m)*8+p
        full = np.empty((128, 128, S3), np.float32)
        for c, o in enumerate(outs):
            o = o.reshape(M_PER_CORE, 128, KS, NB, S2)
            full[c * 16:(c + 1) * 16] = o.transpose(0, 3, 1, 2, 4).reshape(
                16, 128, S3)
        return full.reshape(128, 128, KS, KS, KS)
    full = np.concatenate(outs, axis=0)
    WPAD = KS + 1
    full = full.reshape(128, 128, KS, KS, WPAD)[..., :KS]
    return np.ascontiguousarray(full)


def kernel(k1, k2, shell, factor, _trace=False):
    from concourse.bass_utils import run_bass_kernel_spmd

    nc = _get_nc(MODE)
    in_maps = _make_in_maps(
        np.asarray(k1), np.asarray(k2), np.asarray(shell), np.asarray(factor),
        mode=MODE,
    )
    try:
        res = run_bass_kernel_spmd(
            nc, in_maps, core_ids=list(range(N_CORES)), trace=_trace
        )
    except ModuleNotFoundError:
        res = run_bass_kernel_spmd(
            nc, in_maps, core_ids=list(range(N_CORES)), trace=False
        )
    out = _gather(res.results, MODE)
    if _trace:
        return out, res
    return out
